# revision 70
# baseline (speedup 1.0000x reference)
"""DeepNCM Trainium2 kernel v3: fp8-DoubleRow matmuls + single AllReduce.

Contract: kernel(**inputs) takes FULL unsharded inputs (embeddings
[65536,512] f32, prototypes [1000,512] f32, counter [1000] f32, y_true
[65536] int64) and returns the FULL output [65536,1000] f32.

Per-core plan (NL = 8192 rows, data-parallel over N):
  Host staging (layout/dtype only, no reductions): emb rows fp8 + bf16,
  emb^T fp8, prototypes^T bf16, labels partition-major f32.

  Phase 1: segment sums via one-hot matmuls in fp8 DoubleRow mode (two row
  tiles per matmul); counts via a ones-vector DoubleRow matmul. PSUM pass A
  covers d-chunks 0-2 + counts (8 banks), pass B covers d-chunk 3.
  One-hots built on DVE/GPSIMD (fp8, exact 0/1).

  One AllReduce of [sums(512) ; counts(1)] x 1000 in bf16 (counts are
  small integers => bf16-exact). During the collective: e_sq accumulated
  from bf16 rows (Act square+accum / DVE fused tensor_tensor_reduce, with
  the bf16 row loads issued from the Act DMA queue), and the transposed
  fp8 embeddings are DMA-loaded over the phase-1 row-major fp8 buffer.

  Prototype update: protos2 = 2*(A*p0 + B*sums) per class in transposed
  [d, c] layout; p_sq folded into the phase-2 matmul as a two-term fp8
  decomposition (extra DoubleRow matmul against constant pad weights).

  Phase 2: 2*emb@protos2T - p_sq via 3 DoubleRow fp8 matmuls per
  (tile, class-half); epilogue adds -e_sq as a per-partition bias
  (Act activation / DVE tensor_scalar, split) and emits bf16; the host
  casts bf16 -> f32.
"""

import os
import sys
from contextlib import ExitStack

for _p in ("/opt/trn_rl_repo", "/root/.axon_site/_ro/trn_rl_repo"):
    if os.path.isdir(_p):
        if _p not in sys.path:
            sys.path.insert(0, _p)
        break

import numpy as np
import ml_dtypes

import concourse.bass as bass
import concourse.mybir as mybir
import concourse.tile as tile
from concourse.bass_utils import run_bass_kernel_spmd

N, D, C = 65536, 512, 1000
W = 8                      # cores
NL = N // W                # rows per core
P = 128
KT = NL // P               # 64 row tiles per core
PAIRS = KT // 2            # 32 DoubleRow pairs
DC = D // P                # 4 contraction chunks of d
CH = ((0, 512), (512, 1000))   # class-axis halves (psum bank split)
F32 = mybir.dt.float32
BF16 = mybir.dt.bfloat16
F8 = mybir.dt.float8e4
ALU = mybir.AluOpType
ACTF = mybir.ActivationFunctionType
DR = mybir.MatmulPerfMode.DoubleRow

NP_BF16 = ml_dtypes.bfloat16
NP_F8 = mybir.dt.np(F8)

# Toggled by test.py for profiling runs.
PROFILE = False
TRACE_KWARGS = {}
LAST_RESULT = [None]

_built = [None]


def _split_waits(nc, cap=1):
    """Walrus in this container rejects >1 sync-wait per instruction.
    Move excess waits onto preceding same-engine NOPs (in-order engines,
    so semantics are preserved)."""
    n_new = 0
    for fn in nc.m.functions:
        for bb in fn.blocks:
            new_list = []
            for ins in bb.instructions:
                si = getattr(ins, "sync_info", None)
                if si is not None and si.on_wait and len(si.on_wait) > cap:
                    waits = list(si.on_wait)
                    keep, rest = waits[:cap], waits[cap:]
                    for i in range(0, len(rest), cap):
                        nop = mybir.InstNoOp(
                            name=f"I-waitsplit-{n_new}", ins=[], outs=[]
                        )
                        n_new += 1
                        nop.engine = ins.engine
                        nop.sync_info = mybir.SyncInfo(
                            on_wait=rest[i : i + cap], on_update=[]
                        )
                        new_list.append(nop)
                    si.on_wait = keep
                new_list.append(ins)
            bb.instructions = new_list
    return n_new


# e_sq chunk assignment, interleaved D/A so the streaming input pool
# recycles evenly: DVE (fused tensor_tensor_reduce), Act (square+accum).
E_CHUNK = 4
N_CHUNKS = KT // E_CHUNK
# Act chunks first: Act is idle during phase 1 (DVE owns the one-hot
# feed), and the trailing DVE chunks then land inside the collective
# window once DVE's feed work has drained. Act chunks are loaded from
# the Act DMA queue (self-paced); DVE chunks from the Pool queue so
# they don't serialize behind Act's compute.
E_ORDER = "AAAAAAAAAAAAAAAA"  # per-chunk engine (ttr is not codegen-safe)


def _build():
    nc = bass.Bass()
    emb16_ext = nc.declare_dram_parameter("emb16", [NL, D], BF16, isOutput=False)
    # partition-major fp8 layouts with DoubleRow weight pairs adjacent:
    # emb8[p, t, dc, i, m] = emb[(2t+i)*128+p, dc*128+m]
    # embT8[p, j, nt, i, m] = emb[nt*128+m, (2j+i)*128+p]
    emb8_ext = nc.declare_dram_parameter("emb8", [P, KT * D], F8, isOutput=False)
    embT8_ext = nc.declare_dram_parameter("embT8", [P, KT * D], F8,
                                          isOutput=False)
    yf_ext = nc.declare_dram_parameter("yf", [P, KT], F32, isOutput=False)
    counter_ext = nc.declare_dram_parameter("counter", [C], F32, isOutput=False)
    p0Ts_ext = nc.declare_dram_parameter("p0Ts", [D // W, C], BF16,
                                         isOutput=False)
    out_ext = nc.declare_dram_parameter("out", [NL, C], BF16, isOutput=True)

    with tile.TileContext(nc) as tc, ExitStack() as es:
        cpool = es.enter_context(tc.tile_pool(name="const", bufs=1))
        in_pool = es.enter_context(tc.tile_pool(name="inp", bufs=4))
        sq_pool = es.enter_context(tc.tile_pool(name="sq", bufs=3))
        pr_pool = es.enter_context(tc.tile_pool(name="pr", bufs=2))
        out_pool = es.enter_context(tc.tile_pool(name="outp", bufs=4))
        dram = es.enter_context(tc.tile_pool(name="dram", bufs=1, space="DRAM"))

        # ---- persistent SBUF ----
        iota = cpool.tile([P, C], F32, name="iota")
        nc.gpsimd.iota(
            iota[:], pattern=[[1, C]], base=0, channel_multiplier=0,
            allow_small_or_imprecise_dtypes=True,
        )
        ones_col = cpool.tile([P, 1], BF16, name="onesc")
        nc.vector.memset(ones_col[:], 1.0)
        ones_row = cpool.tile([1, P], BF16, name="onesr")
        nc.vector.memset(ones_row[:], 1.0)
        # dual-fp8 LdWeights requires a 16-wide stationary tile: the counts
        # matmul uses 16 duplicate ones columns (psum rows 1-15 are unused)
        ones2 = cpool.tile([P, 2 * 16], F8, name="ones2")
        nc.vector.memset(ones2[:], 1.0)
        padT = cpool.tile([P, 2 * P], F8, name="padT")
        nc.vector.memset(padT[:], 0.0)
        # (k=0..7, i=0) slots -> weight 1: sums the 8 gathered -p_sq partials
        nc.vector.memset(padT[0:W, 0:P], 1.0)

        y_sb = cpool.tile([P, KT], F32, name="y")
        nc.sync.dma_start(y_sb[:], yf_ext[:])
        ctr_row = cpool.tile([1, C], F32, name="ctr")
        nc.sync.dma_start(ctr_row[:], counter_ext[None, :])

        # emb8 (phase-1 row-major fp8) and embT8 (phase-2 transposed fp8)
        # share one buffer: embT8 is DMA-loaded after the last phase-1
        # matmul reads emb8 (the load overlaps the collective).
        ebuf = cpool.tile([P, KT * D], F8, name="ebuf")
        emb8 = ebuf
        embT8 = ebuf
        LCH = 2  # pairs per load DMA
        for g in range(PAIRS // LCH):
            s = slice(g * LCH * 2 * D, (g + 1) * LCH * 2 * D)
            nc.sync.dma_start(emb8[:, s], emb8_ext[:, s])

        # this core's 64-row shard of prototypes^T (used after the RS)
        p0Ts_sb = cpool.tile([64, C], BF16, name="p0Ts")
        nc.sync.dma_start(p0Ts_sb[:], p0Ts_ext[:, :])

        oh8 = cpool.tile([P, KT * C], F8, name="oh8")
        esq = cpool.tile([P, KT], F32, name="esq")
        esqn = cpool.tile([P, KT], F32, name="esqn")
        sums16 = cpool.tile([P, DC * C], BF16, name="sums16")
        counts16 = cpool.tile([1, C], BF16, name="counts16")
        counts8 = cpool.tile([W, C], BF16, name="counts8")
        p2x = cpool.tile([P, 6 * C], F8, name="p2x")
        nc.vector.memset(p2x[:, 4 * C : 6 * C], 0.0)  # pad rows for psq fold
        A_b = cpool.tile([P, C], BF16, name="Ab")
        B_b = cpool.tile([P, C], BF16, name="Bb")
        t1s = cpool.tile([P, C], BF16, name="t1s")
        t2s = cpool.tile([P, C], BF16, name="t2s")

        emb8r = emb8.rearrange("p (t dc i m) -> p t dc i m", t=PAIRS, dc=DC,
                               i=2)
        oh8r = oh8.rearrange("p (kt c) -> p kt c", kt=KT)
        embT8r = embT8.rearrange("p (j nt i m) -> p j nt i m", j=2, nt=KT,
                                 i=2)
        p2xr = p2x.rearrange("p (b c) -> p b c", b=6)
        sums16r = sums16.rearrange("p (dc c) -> p dc c", dc=DC)
        ones2r = ones2.rearrange("p (i m) -> p i m", i=2)
        padTr = padT.rearrange("p (i m) -> p i m", i=2)

        # ---- one-hots, split DVE/GPSIMD to match the pass-A matmul rate
        # (Pool takes odd tiles below kt=52; its tail would otherwise lag) ----
        for kt in range(KT):
            ohd = oh8[:, kt * C : (kt + 1) * C]
            eng = nc.vector
            eng.tensor_scalar(
                ohd, iota[:], y_sb[:, kt : kt + 1], None, ALU.is_equal
            )

        # ---- phase 1 matmuls: pass A (dc 0-2 + counts), pass B (dc 3) ----
        with tc.tile_pool(name="psA", bufs=1, space="PSUM") as psA:
            sA = [psA.tile([P, 1024], F32, tag=f"sA{dc}", name=f"sA{dc}")
                  for dc in range(3)]
            for t in range(PAIRS):
                st, sp = (t == 0), (t == PAIRS - 1)
                for dc in range(3):
                    for c0, c1 in CH:
                        nc.tensor.matmul(
                            sA[dc][:, c0:c1],
                            emb8r[:, t, dc, :, :],
                            oh8r[:, 2 * t : 2 * t + 2, c0:c1],
                            start=st, stop=sp, perf_mode=DR,
                        )
            # flush pass A (bf16 for the collective) on DVE
            for dc in range(3):
                nc.vector.tensor_copy(
                    out=sums16[:, dc * C : (dc + 1) * C], in_=sA[dc][:, 0:C]
                )

        # counter-only coefficient rows (emitted after the pass-A flush so
        # they don't delay the matmul feed; they run during pass B):
        # rt = 1/(counter+1);  preA = 2*(counter*rt - 1);  preB = 2*rt
        preA = cpool.tile([1, C], F32, name="preA")
        preB = cpool.tile([1, C], F32, name="preB")
        nc.vector.tensor_scalar(preB[:], ctr_row[:], 1.0, None, ALU.add)
        nc.vector.reciprocal(preB[:], preB[:])
        nc.vector.tensor_tensor(out=preA[:], in0=ctr_row[:], in1=preB[:],
                                op=ALU.mult)
        nc.vector.tensor_scalar(preA[:], preA[:], 1.0, None, ALU.subtract)
        nc.vector.tensor_scalar(preA[:], preA[:], 2.0, None, ALU.mult)
        nc.vector.tensor_scalar(preB[:], preB[:], 2.0, None, ALU.mult)

        # ---- ReduceScatter + sharded prototype math + AllGather ----
        # Each core owns one 64-row shard of the d axis. The RS input packs,
        # per shard, [64 sums rows ; local counts] so the reduction delivers
        # global counts to every core alongside its shard. After local
        # prototype math, the AllGather distributes [protos2T-shard ;
        # -p_sq partial] in fp8 (the phase-2 matmul operand, ready to use).
        SH = 65
        cc_rs_in = dram.tile([W * SH, C], BF16, tag="rsi", name="rsi")
        cc_rs_out = dram.tile([SH, C], BF16, tag="rso", name="rso")
        cc_ag_in = dram.tile([SH, C], F8, tag="agi", name="agi")
        cc_ag_out = dram.tile([W * SH, C], F8, tag="ago", name="ago",
                              addr_space="Shared")
        for i in range(6):
            nc.sync.dma_start(
                cc_rs_in[i * SH : i * SH + 64, :],
                sums16r[64 * (i % 2) : 64 * (i % 2) + 64, i // 2, :],
            )

        with tc.tile_pool(name="psB", bufs=1, space="PSUM") as psB:
            sB = psB.tile([P, 1024], F32, tag="sB", name="sB")
            cnt = psB.tile([16, 1024], F32, tag="cnt", name="cnt")
            for t in range(PAIRS):
                st, sp = (t == 0), (t == PAIRS - 1)
                for c0, c1 in CH:
                    nc.tensor.matmul(
                        sB[:, c0:c1],
                        emb8r[:, t, 3, :, :],
                        oh8r[:, 2 * t : 2 * t + 2, c0:c1],
                        start=st, stop=sp, perf_mode=DR,
                    )
                for c0, c1 in CH:
                    nc.tensor.matmul(
                        cnt[0:16, c0:c1],
                        ones2r[:, :, :],
                        oh8r[:, 2 * t : 2 * t + 2, c0:c1],
                        start=st, stop=sp, perf_mode=DR,
                    )
            nc.vector.tensor_copy(out=counts16[:], in_=cnt[0:1, 0:C])
            nc.vector.tensor_copy(out=sums16[:, 3 * C : 4 * C], in_=sB[:, 0:C])

            # replicate local counts across 8 partitions (ones outer product)
            cnt8ps = psB.tile([W, 1024], F32, tag="cnt8", name="cnt8")
            for c0, c1 in CH:
                nc.tensor.matmul(
                    cnt8ps[:, c0:c1], ones_row[0:1, 0:W], counts16[:, c0:c1],
                    start=True, stop=True,
                )
            nc.scalar.copy(counts8[:], cnt8ps[:, 0:C])

        for i in (6, 7):
            nc.sync.dma_start(
                cc_rs_in[i * SH : i * SH + 64, :],
                sums16r[64 * (i % 2) : 64 * (i % 2) + 64, i // 2, :],
            )
        nc.sync.dma_start(
            cc_rs_in.rearrange("(i r) c -> i r c", r=SH)[:, 64:65, :],
            counts8[:, None, :],
        )
        nc.gpsimd.collective_compute(
            "ReduceScatter", ALU.add,
            replica_groups=[list(range(W))],
            ins=[cc_rs_in[:, :]], outs=[cc_rs_out[:, :]],
        )

        # ---- e_sq from bf16 rows + embT8 load, all during the collective.
        # All loads issued from the Act DMA queue: SP parks on the
        # collective's results and the collective freezes the Pool queue
        # for its whole duration, so neither may carry these loads.
        for g in range(N_CHUNKS):
            et = in_pool.tile([P, E_CHUNK * D], BF16, tag="et", name="et")
            dma_eng = nc.scalar
            dma_eng.dma_start(
                et.rearrange("p (j d) -> p j d", j=E_CHUNK),
                emb16_ext.rearrange("(j p) d -> p j d", p=P)[
                    :, g * E_CHUNK : (g + 1) * E_CHUNK, :
                ],
            )
            for j in range(E_CHUNK):
                kt = g * E_CHUNK + j
                ets = et[:, j * D : (j + 1) * D]
                scr = sq_pool.tile([P, D], BF16, tag="scr", name="scr")
                eng = E_ORDER[g]
                if eng == "D":
                    nc.vector.tensor_tensor_reduce(
                        out=scr[:], in0=ets, in1=ets, scale=1.0, scalar=0.0,
                        op0=ALU.mult, op1=ALU.add,
                        accum_out=esq[:, kt : kt + 1],
                    )
                else:
                    nc.scalar.activation(
                        scr[:], ets, ACTF.Square,
                        accum_out=esq[:, kt : kt + 1],
                    )
        # transposed fp8 embeddings overwrite emb8 (WAR on phase-1 matmuls);
        # issued from the Act queue (after its e_sq work) so the transfer
        # lands inside the collective window without delaying the staging.
        nc.scalar.dma_start(embT8[:, :], embT8_ext[:, :])

        # RS results back (SP queue; parks until the collective finishes).
        # Counts row first: the coefficient rows depend only on counts.
        shard_s = cpool.tile([64, C], BF16, name="shards")
        nc.sync.dma_start(counts16[:], cc_rs_out[64:65, :])
        nc.sync.dma_start(shard_s[:], cc_rs_out[0:64, :])

        # ---- negate e_sq (phase-2 bias) ----
        nc.vector.tensor_scalar(esqn[:], esq[:], -1.0, None, ALU.mult)

        # ---- per-class coefficients:  protos2 = 2A*p0 + 2B*sums
        # 2A = 2 + rep*preA ;  2B = rep * (preB * 1/max(counts,1))
        countsf = cpool.tile([1, C], F32, name="countsf")
        rep = cpool.tile([1, C], F32, name="rep")
        tmp1 = cpool.tile([1, C], F32, name="tmp1")
        tmp2 = cpool.tile([1, C], F32, name="tmp2")
        A_row = cpool.tile([1, C], BF16, name="Arow")
        B_row = cpool.tile([1, C], BF16, name="Brow")
        nc.vector.tensor_copy(out=countsf[:], in_=counts16[:])
        nc.vector.tensor_scalar(rep[:], countsf[:], 0.0, None, ALU.is_gt)
        nc.vector.tensor_scalar(tmp1[:], countsf[:], 1.0, None, ALU.max)
        nc.vector.reciprocal(tmp1[:], tmp1[:])
        nc.vector.tensor_tensor(out=tmp2[:], in0=preB[:], in1=tmp1[:],
                                op=ALU.mult)
        nc.vector.tensor_tensor(out=B_row[:], in0=tmp2[:], in1=rep[:],
                                op=ALU.mult)
        nc.vector.tensor_tensor(out=tmp2[:], in0=preA[:], in1=rep[:],
                                op=ALU.mult)
        nc.vector.tensor_scalar(A_row[:], tmp2[:], 2.0, None, ALU.add)

        with tc.tile_pool(name="psM", bufs=1, space="PSUM") as psM:
            # broadcast A,B down partitions via ones outer product
            for row, dstb in ((B_row, B_b), (A_row, A_b)):
                ob = psM.tile([64, 1024], F32, tag="ob", bufs=2, name="ob")
                for c0, c1 in CH:
                    nc.tensor.matmul(
                        ob[:, c0:c1], ones_row[0:1, 0:64], row[:, c0:c1],
                        start=True, stop=True,
                    )
                nc.scalar.copy(dstb[0:64, :], ob[:, 0:C])

            # this core's 64-row protos2T shard (fp8) + its -p_sq partial
            p2sh = cpool.tile([64, C], F8, name="p2sh")
            nc.vector.tensor_tensor(out=t1s[0:64, :], in0=p0Ts_sb[:],
                                    in1=A_b[0:64, :], op=ALU.mult)
            nc.vector.tensor_tensor(out=t2s[0:64, :], in0=shard_s[:],
                                    in1=B_b[0:64, :], op=ALU.mult)
            nc.vector.tensor_tensor(out=p2sh[:], in0=t1s[0:64, :],
                                    in1=t2s[0:64, :], op=ALU.add)
            sqs = pr_pool.tile([64, C], BF16, tag="sqs", name="sqs")
            nc.scalar.activation(sqs[:], p2sh[:], ACTF.Square)
            psqps = psM.tile([1, 1024], F32, tag="psq", name="psq")
            for c0, c1 in CH:
                nc.tensor.matmul(
                    psqps[:, c0:c1], ones_col[0:64, :], sqs[:, c0:c1],
                    start=True, stop=True,
                )
            psqn = tmp1  # coefficient scratch rows are dead by now
            nc.scalar.mul(psqn[:], psqps[:, 0:C], -0.25)
            psq8 = cpool.tile([1, C], F8, name="psq8")
            nc.vector.tensor_copy(out=psq8[:], in_=psqn[:])

        # AllGather [protos2T-shard ; -p_sq partial] (fp8)
        nc.sync.dma_start(cc_ag_in[0:64, :], p2sh[:])
        nc.sync.dma_start(cc_ag_in[64:65, :], psq8[:])
        nc.gpsimd.collective_compute(
            "AllGather", ALU.bypass,
            replica_groups=[list(range(W))],
            ins=[cc_ag_in[:, :]], outs=[cc_ag_out[:, :]],
        )
        # assemble the phase-2 operand: p2x[p, dc, :] holds d = dc*128+p,
        # and the 8 -p_sq partials land on partitions 0-7 of pad block 4
        for dc in range(DC):
            nc.sync.dma_start(
                p2xr[0:64, dc, :], cc_ag_out[2 * dc * SH : 2 * dc * SH + 64, :]
            )
            nc.sync.dma_start(
                p2xr[64:P, dc, :],
                cc_ag_out[(2 * dc + 1) * SH : (2 * dc + 1) * SH + 64, :],
            )
        nc.sync.dma_start(
            p2x[0:W, 4 * C : 5 * C],
            cc_ag_out.rearrange("(i r) c -> i r c", r=SH)[:, 64, :],
        )

        # ---- phase 2: out = 2*emb@protos2T - p_sq - e_sq ----
        OB = 2  # output tiles per DMA
        with tc.tile_pool(name="ps2", bufs=4, space="PSUM") as ps2:
            for nt in range(KT):
                if nt % OB == 0:
                    ot = out_pool.tile([P, OB * C], BF16, tag="ot", name="ot")
                # per-half psum groups: each class-half closes and drains
                # independently, halving the psum-token recycle latency
                crh = [ps2.tile([P, 512], F32, tag=f"cr{ci}", name=f"cr{ci}")
                       for ci in range(2)]
                ots = ot[:, (nt % OB) * C : (nt % OB + 1) * C]
                for ci, (c0, c1) in enumerate(CH):
                    cw = c1 - c0
                    cr = crh[ci]
                    for j in range(2):
                        nc.tensor.matmul(
                            cr[:, 0:cw],
                            embT8r[:, j, nt, :, :],
                            p2xr[:, 2 * j : 2 * j + 2, c0:c1],
                            start=(j == 0), stop=False, perf_mode=DR,
                        )
                    nc.tensor.matmul(
                        cr[:, 0:cw],
                        padTr[:, :, :],
                        p2xr[:, 4:6, c0:c1],
                        start=False, stop=True, perf_mode=DR,
                    )
                    # epilogue: Act drains half A, DVE half B, in parallel
                    if ci == 0:
                        nc.scalar.activation(
                            ots[:, c0:c1], cr[:, 0:cw], ACTF.Identity,
                            bias=esqn[:, nt : nt + 1], scale=1.0,
                        )
                    else:
                        nc.vector.tensor_scalar(
                            ots[:, c0:c1], cr[:, 0:cw],
                            esqn[:, nt : nt + 1], None, ALU.add
                        )
                if nt % OB == OB - 1:
                    nc.sync.dma_start(
                        out_ext.rearrange("(j p) c -> p j c", p=P)[
                            :, nt - OB + 1 : nt + 1, :
                        ],
                        ot.rearrange("p (j c) -> p j c", j=OB),
                    )

    _split_waits(nc)
    return nc


def kernel(embeddings, prototypes, counter, y_true):
    embeddings = np.ascontiguousarray(np.asarray(embeddings, dtype=np.float32))
    prototypes = np.ascontiguousarray(np.asarray(prototypes, dtype=np.float32))
    counter_f = np.ascontiguousarray(np.asarray(counter, dtype=np.float32))
    y = np.asarray(y_true)

    if _built[0] is None:
        _built[0] = _build()
    nc = _built[0]

    p0T16 = np.ascontiguousarray(prototypes.T).astype(NP_BF16)  # [512, 1000]
    in_maps = []
    for i in range(W):
        sl = slice(i * NL, (i + 1) * NL)
        e = embeddings[sl]
        e8 = e.astype(NP_F8)
        # emb8[p, t, dc, i, m]; embT8[p, j, nt, i, m] (DoubleRow pairs adjacent)
        e8v = e8.reshape(PAIRS, 2, P, DC, P)
        e8p = np.ascontiguousarray(e8v.transpose(2, 0, 3, 1, 4)).reshape(P, KT * D)
        eTv = np.ascontiguousarray(e8.T).reshape(2, 2, P, KT, P)
        eTp = np.ascontiguousarray(eTv.transpose(2, 0, 3, 1, 4)).reshape(P, KT * D)
        y_loc = y[sl].astype(np.float32)
        yf = np.ascontiguousarray(y_loc.reshape(KT, P).T)
        in_maps.append(
            {
                "emb16": e.astype(NP_BF16),
                "emb8": e8p,
                "embT8": eTp,
                "yf": yf,
                "counter": counter_f,
                "p0Ts": np.ascontiguousarray(p0T16[i * (D // W) : (i + 1) * (D // W)]),
            }
        )

    res = run_bass_kernel_spmd(
        nc, in_maps, list(range(W)), trace=PROFILE, **TRACE_KWARGS
    )
    LAST_RESULT[0] = res
    out = np.concatenate(
        [np.asarray(res.results[i]["out"]) for i in range(W)], axis=0
    )
    return out.astype(np.float32)


# revision 71
# speedup vs baseline: 1.0164x; 1.0164x over previous
"""DeepNCM Trainium2 kernel v3: fp8-DoubleRow matmuls + single AllReduce.

Contract: kernel(**inputs) takes FULL unsharded inputs (embeddings
[65536,512] f32, prototypes [1000,512] f32, counter [1000] f32, y_true
[65536] int64) and returns the FULL output [65536,1000] f32.

Per-core plan (NL = 8192 rows, data-parallel over N):
  Host staging (layout/dtype only, no reductions): emb rows fp8 + bf16,
  emb^T fp8, prototypes^T bf16, labels partition-major f32.

  Phase 1: segment sums via one-hot matmuls in fp8 DoubleRow mode (two row
  tiles per matmul); counts via a ones-vector DoubleRow matmul. PSUM pass A
  covers d-chunks 0-2 + counts (8 banks), pass B covers d-chunk 3.
  One-hots built on DVE/GPSIMD (fp8, exact 0/1).

  One AllReduce of [sums(512) ; counts(1)] x 1000 in bf16 (counts are
  small integers => bf16-exact). During the collective: e_sq accumulated
  from bf16 rows (Act square+accum / DVE fused tensor_tensor_reduce, with
  the bf16 row loads issued from the Act DMA queue), and the transposed
  fp8 embeddings are DMA-loaded over the phase-1 row-major fp8 buffer.

  Prototype update: protos2 = 2*(A*p0 + B*sums) per class in transposed
  [d, c] layout; p_sq folded into the phase-2 matmul as a two-term fp8
  decomposition (extra DoubleRow matmul against constant pad weights).

  Phase 2: 2*emb@protos2T - p_sq via 3 DoubleRow fp8 matmuls per
  (tile, class-half); epilogue adds -e_sq as a per-partition bias
  (Act activation / DVE tensor_scalar, split) and emits bf16; the host
  casts bf16 -> f32.
"""

import os
import sys
from contextlib import ExitStack

for _p in ("/opt/trn_rl_repo", "/root/.axon_site/_ro/trn_rl_repo"):
    if os.path.isdir(_p):
        if _p not in sys.path:
            sys.path.insert(0, _p)
        break

import numpy as np
import ml_dtypes

import concourse.bass as bass
import concourse.mybir as mybir
import concourse.tile as tile
from concourse.bass_utils import run_bass_kernel_spmd

N, D, C = 65536, 512, 1000
W = 8                      # cores
NL = N // W                # rows per core
P = 128
KT = NL // P               # 64 row tiles per core
PAIRS = KT // 2            # 32 DoubleRow pairs
DC = D // P                # 4 contraction chunks of d
CH = ((0, 512), (512, 1000))   # class-axis halves (psum bank split)
F32 = mybir.dt.float32
BF16 = mybir.dt.bfloat16
F8 = mybir.dt.float8e4
ALU = mybir.AluOpType
ACTF = mybir.ActivationFunctionType
DR = mybir.MatmulPerfMode.DoubleRow

NP_BF16 = ml_dtypes.bfloat16
NP_F8 = mybir.dt.np(F8)

# Toggled by test.py for profiling runs.
PROFILE = False
TRACE_KWARGS = {}
LAST_RESULT = [None]

_built = [None]


def _split_waits(nc, cap=1):
    """Walrus in this container rejects >1 sync-wait per instruction.
    Move excess waits onto preceding same-engine NOPs (in-order engines,
    so semantics are preserved)."""
    n_new = 0
    for fn in nc.m.functions:
        for bb in fn.blocks:
            new_list = []
            for ins in bb.instructions:
                si = getattr(ins, "sync_info", None)
                if si is not None and si.on_wait and len(si.on_wait) > cap:
                    waits = list(si.on_wait)
                    keep, rest = waits[:cap], waits[cap:]
                    for i in range(0, len(rest), cap):
                        nop = mybir.InstNoOp(
                            name=f"I-waitsplit-{n_new}", ins=[], outs=[]
                        )
                        n_new += 1
                        nop.engine = ins.engine
                        nop.sync_info = mybir.SyncInfo(
                            on_wait=rest[i : i + cap], on_update=[]
                        )
                        new_list.append(nop)
                    si.on_wait = keep
                new_list.append(ins)
            bb.instructions = new_list
    return n_new


# e_sq chunk assignment, interleaved D/A so the streaming input pool
# recycles evenly: DVE (fused tensor_tensor_reduce), Act (square+accum).
E_CHUNK = 4
N_CHUNKS = KT // E_CHUNK
# Act chunks first: Act is idle during phase 1 (DVE owns the one-hot
# feed), and the trailing DVE chunks then land inside the collective
# window once DVE's feed work has drained. Act chunks are loaded from
# the Act DMA queue (self-paced); DVE chunks from the Pool queue so
# they don't serialize behind Act's compute.
E_ORDER = "AAAAAAAAAAAAAAAA"  # per-chunk engine (ttr is not codegen-safe)


def _build():
    nc = bass.Bass()
    emb16_ext = nc.declare_dram_parameter("emb16", [NL, D], BF16, isOutput=False)
    # partition-major fp8 layouts with DoubleRow weight pairs adjacent:
    # emb8[p, t, dc, i, m] = emb[(2t+i)*128+p, dc*128+m]
    # embT8[p, j, nt, i, m] = emb[nt*128+m, (2j+i)*128+p]
    emb8_ext = nc.declare_dram_parameter("emb8", [P, KT * D], F8, isOutput=False)
    embT8_ext = nc.declare_dram_parameter("embT8", [P, KT * D], F8,
                                          isOutput=False)
    yf_ext = nc.declare_dram_parameter("yf", [P, KT], F32, isOutput=False)
    counter_ext = nc.declare_dram_parameter("counter", [C], F32, isOutput=False)
    p0Ts_ext = nc.declare_dram_parameter("p0Ts", [D // W, C], BF16,
                                         isOutput=False)
    out_ext = nc.declare_dram_parameter("out", [NL, C], BF16, isOutput=True)

    with tile.TileContext(nc) as tc, ExitStack() as es:
        cpool = es.enter_context(tc.tile_pool(name="const", bufs=1))
        in_pool = es.enter_context(tc.tile_pool(name="inp", bufs=4))
        sq_pool = es.enter_context(tc.tile_pool(name="sq", bufs=3))
        pr_pool = es.enter_context(tc.tile_pool(name="pr", bufs=2))
        out_pool = es.enter_context(tc.tile_pool(name="outp", bufs=4))
        dram = es.enter_context(tc.tile_pool(name="dram", bufs=1, space="DRAM"))

        # ---- persistent SBUF ----
        iota = cpool.tile([P, C], F32, name="iota")
        nc.gpsimd.iota(
            iota[:], pattern=[[1, C]], base=0, channel_multiplier=0,
            allow_small_or_imprecise_dtypes=True,
        )
        ones_col = cpool.tile([P, 1], BF16, name="onesc")
        nc.vector.memset(ones_col[:], 1.0)
        ones_row = cpool.tile([1, P], BF16, name="onesr")
        nc.vector.memset(ones_row[:], 1.0)
        # dual-fp8 LdWeights requires a 16-wide stationary tile: the counts
        # matmul uses 16 duplicate ones columns (psum rows 1-15 are unused)
        ones2 = cpool.tile([P, 2 * 16], F8, name="ones2")
        nc.vector.memset(ones2[:], 1.0)
        padT = cpool.tile([P, 2 * P], F8, name="padT")
        nc.vector.memset(padT[:], 0.0)
        # (k=0..7, i=0) slots -> weight 1: sums the 8 gathered -p_sq partials
        nc.vector.memset(padT[0:W, 0:P], 1.0)

        y_sb = cpool.tile([P, KT], F32, name="y")
        nc.sync.dma_start(y_sb[:], yf_ext[:])
        ctr_row = cpool.tile([1, C], F32, name="ctr")
        nc.sync.dma_start(ctr_row[:], counter_ext[None, :])

        # emb8 (phase-1 row-major fp8) and embT8 (phase-2 transposed fp8)
        # share one buffer: embT8 is DMA-loaded after the last phase-1
        # matmul reads emb8 (the load overlaps the collective).
        ebuf = cpool.tile([P, KT * D], F8, name="ebuf")
        emb8 = ebuf
        embT8 = ebuf
        LCH = 2  # pairs per load DMA
        for g in range(PAIRS // LCH):
            s = slice(g * LCH * 2 * D, (g + 1) * LCH * 2 * D)
            nc.sync.dma_start(emb8[:, s], emb8_ext[:, s])

        # this core's 64-row shard of prototypes^T (used after the RS)
        p0Ts_sb = cpool.tile([64, C], BF16, name="p0Ts")
        nc.sync.dma_start(p0Ts_sb[:], p0Ts_ext[:, :])

        oh8 = cpool.tile([P, KT * C], F8, name="oh8")
        esq = cpool.tile([P, KT], F32, name="esq")
        esqn = cpool.tile([P, KT], F32, name="esqn")
        sums16 = cpool.tile([P, DC * C], BF16, name="sums16")
        counts16 = cpool.tile([1, C], BF16, name="counts16")
        counts8 = cpool.tile([W, C], BF16, name="counts8")
        p2x = cpool.tile([P, 6 * C], F8, name="p2x")
        nc.vector.memset(p2x[:, 4 * C : 6 * C], 0.0)  # pad rows for psq fold
        A_b = cpool.tile([P, C], BF16, name="Ab")
        B_b = cpool.tile([P, C], BF16, name="Bb")
        t1s = cpool.tile([P, C], BF16, name="t1s")
        t2s = cpool.tile([P, C], BF16, name="t2s")

        emb8r = emb8.rearrange("p (t dc i m) -> p t dc i m", t=PAIRS, dc=DC,
                               i=2)
        oh8r = oh8.rearrange("p (kt c) -> p kt c", kt=KT)
        embT8r = embT8.rearrange("p (j nt i m) -> p j nt i m", j=2, nt=KT,
                                 i=2)
        p2xr = p2x.rearrange("p (b c) -> p b c", b=6)
        sums16r = sums16.rearrange("p (dc c) -> p dc c", dc=DC)
        ones2r = ones2.rearrange("p (i m) -> p i m", i=2)
        padTr = padT.rearrange("p (i m) -> p i m", i=2)

        # ---- one-hots, split DVE/GPSIMD to match the pass-A matmul rate
        # (Pool takes odd tiles below kt=52; its tail would otherwise lag) ----
        for kt in range(KT):
            ohd = oh8[:, kt * C : (kt + 1) * C]
            eng = nc.gpsimd if (kt % 2 == 1 and kt < 52) else nc.vector
            eng.tensor_scalar(
                ohd, iota[:], y_sb[:, kt : kt + 1], None, ALU.is_equal
            )

        # ---- phase 1 matmuls: pass A (dc 0-2 + counts), pass B (dc 3) ----
        with tc.tile_pool(name="psA", bufs=1, space="PSUM") as psA:
            sA = [psA.tile([P, 1024], F32, tag=f"sA{dc}", name=f"sA{dc}")
                  for dc in range(3)]
            for t in range(PAIRS):
                st, sp = (t == 0), (t == PAIRS - 1)
                for dc in range(3):
                    for c0, c1 in CH:
                        nc.tensor.matmul(
                            sA[dc][:, c0:c1],
                            emb8r[:, t, dc, :, :],
                            oh8r[:, 2 * t : 2 * t + 2, c0:c1],
                            start=st, stop=sp, perf_mode=DR,
                        )
            # flush pass A (bf16 for the collective) on DVE
            for dc in range(3):
                nc.vector.tensor_copy(
                    out=sums16[:, dc * C : (dc + 1) * C], in_=sA[dc][:, 0:C]
                )

        # counter-only coefficient rows (emitted after the pass-A flush so
        # they don't delay the matmul feed; they run during pass B):
        # rt = 1/(counter+1);  preA = 2*(counter*rt - 1);  preB = 2*rt
        preA = cpool.tile([1, C], F32, name="preA")
        preB = cpool.tile([1, C], F32, name="preB")
        nc.vector.tensor_scalar(preB[:], ctr_row[:], 1.0, None, ALU.add)
        nc.vector.reciprocal(preB[:], preB[:])
        nc.vector.tensor_tensor(out=preA[:], in0=ctr_row[:], in1=preB[:],
                                op=ALU.mult)
        nc.vector.tensor_scalar(preA[:], preA[:], 1.0, None, ALU.subtract)
        nc.vector.tensor_scalar(preA[:], preA[:], 2.0, None, ALU.mult)
        nc.vector.tensor_scalar(preB[:], preB[:], 2.0, None, ALU.mult)

        # ---- ReduceScatter + sharded prototype math + AllGather ----
        # Each core owns one 64-row shard of the d axis. The RS input packs,
        # per shard, [64 sums rows ; local counts] so the reduction delivers
        # global counts to every core alongside its shard. After local
        # prototype math, the AllGather distributes [protos2T-shard ;
        # -p_sq partial] in fp8 (the phase-2 matmul operand, ready to use).
        SH = 65
        cc_rs_in = dram.tile([W * SH, C], BF16, tag="rsi", name="rsi")
        cc_rs_out = dram.tile([SH, C], BF16, tag="rso", name="rso")
        cc_ag_in = dram.tile([SH, C], F8, tag="agi", name="agi")
        cc_ag_out = dram.tile([W * SH, C], F8, tag="ago", name="ago",
                              addr_space="Shared")
        for i in range(6):
            nc.sync.dma_start(
                cc_rs_in[i * SH : i * SH + 64, :],
                sums16r[64 * (i % 2) : 64 * (i % 2) + 64, i // 2, :],
            )

        with tc.tile_pool(name="psB", bufs=1, space="PSUM") as psB:
            sB = psB.tile([P, 1024], F32, tag="sB", name="sB")
            cnt = psB.tile([16, 1024], F32, tag="cnt", name="cnt")
            for t in range(PAIRS):
                st, sp = (t == 0), (t == PAIRS - 1)
                for c0, c1 in CH:
                    nc.tensor.matmul(
                        sB[:, c0:c1],
                        emb8r[:, t, 3, :, :],
                        oh8r[:, 2 * t : 2 * t + 2, c0:c1],
                        start=st, stop=sp, perf_mode=DR,
                    )
                for c0, c1 in CH:
                    nc.tensor.matmul(
                        cnt[0:16, c0:c1],
                        ones2r[:, :, :],
                        oh8r[:, 2 * t : 2 * t + 2, c0:c1],
                        start=st, stop=sp, perf_mode=DR,
                    )
            nc.vector.tensor_copy(out=counts16[:], in_=cnt[0:1, 0:C])
            nc.vector.tensor_copy(out=sums16[:, 3 * C : 4 * C], in_=sB[:, 0:C])

            # replicate local counts across 8 partitions (ones outer product)
            cnt8ps = psB.tile([W, 1024], F32, tag="cnt8", name="cnt8")
            for c0, c1 in CH:
                nc.tensor.matmul(
                    cnt8ps[:, c0:c1], ones_row[0:1, 0:W], counts16[:, c0:c1],
                    start=True, stop=True,
                )
            nc.scalar.copy(counts8[:], cnt8ps[:, 0:C])

        for i in (6, 7):
            nc.sync.dma_start(
                cc_rs_in[i * SH : i * SH + 64, :],
                sums16r[64 * (i % 2) : 64 * (i % 2) + 64, i // 2, :],
            )
        nc.sync.dma_start(
            cc_rs_in.rearrange("(i r) c -> i r c", r=SH)[:, 64:65, :],
            counts8[:, None, :],
        )
        nc.gpsimd.collective_compute(
            "ReduceScatter", ALU.add,
            replica_groups=[list(range(W))],
            ins=[cc_rs_in[:, :]], outs=[cc_rs_out[:, :]],
        )

        # ---- e_sq from bf16 rows + embT8 load, all during the collective.
        # All loads issued from the Act DMA queue: SP parks on the
        # collective's results and the collective freezes the Pool queue
        # for its whole duration, so neither may carry these loads.
        for g in range(N_CHUNKS):
            et = in_pool.tile([P, E_CHUNK * D], BF16, tag="et", name="et")
            dma_eng = nc.scalar
            dma_eng.dma_start(
                et.rearrange("p (j d) -> p j d", j=E_CHUNK),
                emb16_ext.rearrange("(j p) d -> p j d", p=P)[
                    :, g * E_CHUNK : (g + 1) * E_CHUNK, :
                ],
            )
            for j in range(E_CHUNK):
                kt = g * E_CHUNK + j
                ets = et[:, j * D : (j + 1) * D]
                scr = sq_pool.tile([P, D], BF16, tag="scr", name="scr")
                eng = E_ORDER[g]
                if eng == "D":
                    nc.vector.tensor_tensor_reduce(
                        out=scr[:], in0=ets, in1=ets, scale=1.0, scalar=0.0,
                        op0=ALU.mult, op1=ALU.add,
                        accum_out=esq[:, kt : kt + 1],
                    )
                else:
                    nc.scalar.activation(
                        scr[:], ets, ACTF.Square,
                        accum_out=esq[:, kt : kt + 1],
                    )
        # transposed fp8 embeddings overwrite emb8 (WAR on phase-1 matmuls);
        # issued from the Act queue (after its e_sq work) so the transfer
        # lands inside the collective window without delaying the staging.
        nc.scalar.dma_start(embT8[:, :], embT8_ext[:, :])

        # RS results back (SP queue; parks until the collective finishes).
        # Counts row first: the coefficient rows depend only on counts.
        shard_s = cpool.tile([64, C], BF16, name="shards")
        nc.sync.dma_start(counts16[:], cc_rs_out[64:65, :])
        nc.sync.dma_start(shard_s[:], cc_rs_out[0:64, :])

        # ---- negate e_sq (phase-2 bias) ----
        nc.vector.tensor_scalar(esqn[:], esq[:], -1.0, None, ALU.mult)

        # ---- per-class coefficients:  protos2 = 2A*p0 + 2B*sums
        # 2A = 2 + rep*preA ;  2B = rep * (preB * 1/max(counts,1))
        countsf = cpool.tile([1, C], F32, name="countsf")
        rep = cpool.tile([1, C], F32, name="rep")
        tmp1 = cpool.tile([1, C], F32, name="tmp1")
        tmp2 = cpool.tile([1, C], F32, name="tmp2")
        A_row = cpool.tile([1, C], BF16, name="Arow")
        B_row = cpool.tile([1, C], BF16, name="Brow")
        nc.vector.tensor_copy(out=countsf[:], in_=counts16[:])
        nc.vector.tensor_scalar(rep[:], countsf[:], 0.0, None, ALU.is_gt)
        nc.vector.tensor_scalar(tmp1[:], countsf[:], 1.0, None, ALU.max)
        nc.vector.reciprocal(tmp1[:], tmp1[:])
        nc.vector.tensor_tensor(out=tmp2[:], in0=preB[:], in1=tmp1[:],
                                op=ALU.mult)
        nc.vector.tensor_tensor(out=B_row[:], in0=tmp2[:], in1=rep[:],
                                op=ALU.mult)
        nc.vector.tensor_tensor(out=tmp2[:], in0=preA[:], in1=rep[:],
                                op=ALU.mult)
        nc.vector.tensor_scalar(A_row[:], tmp2[:], 2.0, None, ALU.add)

        with tc.tile_pool(name="psM", bufs=1, space="PSUM") as psM:
            # broadcast A,B down partitions via ones outer product
            for row, dstb in ((B_row, B_b), (A_row, A_b)):
                ob = psM.tile([64, 1024], F32, tag="ob", bufs=2, name="ob")
                for c0, c1 in CH:
                    nc.tensor.matmul(
                        ob[:, c0:c1], ones_row[0:1, 0:64], row[:, c0:c1],
                        start=True, stop=True,
                    )
                nc.scalar.copy(dstb[0:64, :], ob[:, 0:C])

            # this core's 64-row protos2T shard (fp8) + its -p_sq partial
            p2sh = cpool.tile([64, C], F8, name="p2sh")
            nc.vector.tensor_tensor(out=t1s[0:64, :], in0=p0Ts_sb[:],
                                    in1=A_b[0:64, :], op=ALU.mult)
            nc.vector.tensor_tensor(out=t2s[0:64, :], in0=shard_s[:],
                                    in1=B_b[0:64, :], op=ALU.mult)
            nc.vector.tensor_tensor(out=p2sh[:], in0=t1s[0:64, :],
                                    in1=t2s[0:64, :], op=ALU.add)
            sqs = pr_pool.tile([64, C], BF16, tag="sqs", name="sqs")
            nc.scalar.activation(sqs[:], p2sh[:], ACTF.Square)
            psqps = psM.tile([1, 1024], F32, tag="psq", name="psq")
            for c0, c1 in CH:
                nc.tensor.matmul(
                    psqps[:, c0:c1], ones_col[0:64, :], sqs[:, c0:c1],
                    start=True, stop=True,
                )
            psqn = tmp1  # coefficient scratch rows are dead by now
            nc.scalar.mul(psqn[:], psqps[:, 0:C], -0.25)
            psq8 = cpool.tile([1, C], F8, name="psq8")
            nc.vector.tensor_copy(out=psq8[:], in_=psqn[:])

        # AllGather [protos2T-shard ; -p_sq partial] (fp8)
        nc.sync.dma_start(cc_ag_in[0:64, :], p2sh[:])
        nc.sync.dma_start(cc_ag_in[64:65, :], psq8[:])
        nc.gpsimd.collective_compute(
            "AllGather", ALU.bypass,
            replica_groups=[list(range(W))],
            ins=[cc_ag_in[:, :]], outs=[cc_ag_out[:, :]],
        )
        # assemble the phase-2 operand: p2x[p, dc, :] holds d = dc*128+p,
        # and the 8 -p_sq partials land on partitions 0-7 of pad block 4
        for dc in range(DC):
            nc.sync.dma_start(
                p2xr[0:64, dc, :], cc_ag_out[2 * dc * SH : 2 * dc * SH + 64, :]
            )
            nc.sync.dma_start(
                p2xr[64:P, dc, :],
                cc_ag_out[(2 * dc + 1) * SH : (2 * dc + 1) * SH + 64, :],
            )
        nc.sync.dma_start(
            p2x[0:W, 4 * C : 5 * C],
            cc_ag_out.rearrange("(i r) c -> i r c", r=SH)[:, 64, :],
        )

        # ---- phase 2: out = 2*emb@protos2T - p_sq - e_sq ----
        OB = 2  # output tiles per DMA
        with tc.tile_pool(name="ps2", bufs=4, space="PSUM") as ps2:
            for nt in range(KT):
                if nt % OB == 0:
                    ot = out_pool.tile([P, OB * C], BF16, tag="ot", name="ot")
                # per-half psum groups: each class-half closes and drains
                # independently, halving the psum-token recycle latency
                crh = [ps2.tile([P, 512], F32, tag=f"cr{ci}", name=f"cr{ci}")
                       for ci in range(2)]
                ots = ot[:, (nt % OB) * C : (nt % OB + 1) * C]
                for ci, (c0, c1) in enumerate(CH):
                    cw = c1 - c0
                    cr = crh[ci]
                    for j in range(2):
                        nc.tensor.matmul(
                            cr[:, 0:cw],
                            embT8r[:, j, nt, :, :],
                            p2xr[:, 2 * j : 2 * j + 2, c0:c1],
                            start=(j == 0), stop=False, perf_mode=DR,
                        )
                    nc.tensor.matmul(
                        cr[:, 0:cw],
                        padTr[:, :, :],
                        p2xr[:, 4:6, c0:c1],
                        start=False, stop=True, perf_mode=DR,
                    )
                    # epilogue: Act drains half A, DVE half B, in parallel
                    if ci == 0:
                        nc.scalar.activation(
                            ots[:, c0:c1], cr[:, 0:cw], ACTF.Identity,
                            bias=esqn[:, nt : nt + 1], scale=1.0,
                        )
                    else:
                        nc.vector.tensor_scalar(
                            ots[:, c0:c1], cr[:, 0:cw],
                            esqn[:, nt : nt + 1], None, ALU.add
                        )
                if nt % OB == OB - 1:
                    nc.sync.dma_start(
                        out_ext.rearrange("(j p) c -> p j c", p=P)[
                            :, nt - OB + 1 : nt + 1, :
                        ],
                        ot.rearrange("p (j c) -> p j c", j=OB),
                    )

    _split_waits(nc)
    return nc


def kernel(embeddings, prototypes, counter, y_true):
    embeddings = np.ascontiguousarray(np.asarray(embeddings, dtype=np.float32))
    prototypes = np.ascontiguousarray(np.asarray(prototypes, dtype=np.float32))
    counter_f = np.ascontiguousarray(np.asarray(counter, dtype=np.float32))
    y = np.asarray(y_true)

    if _built[0] is None:
        _built[0] = _build()
    nc = _built[0]

    p0T16 = np.ascontiguousarray(prototypes.T).astype(NP_BF16)  # [512, 1000]
    in_maps = []
    for i in range(W):
        sl = slice(i * NL, (i + 1) * NL)
        e = embeddings[sl]
        e8 = e.astype(NP_F8)
        # emb8[p, t, dc, i, m]; embT8[p, j, nt, i, m] (DoubleRow pairs adjacent)
        e8v = e8.reshape(PAIRS, 2, P, DC, P)
        e8p = np.ascontiguousarray(e8v.transpose(2, 0, 3, 1, 4)).reshape(P, KT * D)
        eTv = np.ascontiguousarray(e8.T).reshape(2, 2, P, KT, P)
        eTp = np.ascontiguousarray(eTv.transpose(2, 0, 3, 1, 4)).reshape(P, KT * D)
        y_loc = y[sl].astype(np.float32)
        yf = np.ascontiguousarray(y_loc.reshape(KT, P).T)
        in_maps.append(
            {
                "emb16": e.astype(NP_BF16),
                "emb8": e8p,
                "embT8": eTp,
                "yf": yf,
                "counter": counter_f,
                "p0Ts": np.ascontiguousarray(p0T16[i * (D // W) : (i + 1) * (D // W)]),
            }
        )

    res = run_bass_kernel_spmd(
        nc, in_maps, list(range(W)), trace=PROFILE, **TRACE_KWARGS
    )
    LAST_RESULT[0] = res
    out = np.concatenate(
        [np.asarray(res.results[i]["out"]) for i in range(W)], axis=0
    )
    return out.astype(np.float32)


# revision 73
# speedup vs baseline: 1.0643x; 1.0472x over previous
"""DeepNCM Trainium2 kernel: fp8-DoubleRow matmuls + ReduceScatter/AllGather.

Contract: kernel(**inputs) takes FULL unsharded inputs (embeddings
[65536,512] f32, prototypes [1000,512] f32, counter [1000] f32, y_true
[65536] int64) and returns the FULL output [65536,1000] f32.

Per-core plan (NL = 8192 rows, data-parallel over N):
  Host staging (layout/dtype only, no reductions): emb rows fp8 + bf16,
  emb^T fp8, prototypes^T bf16, labels partition-major f32.

  Phase 1: segment sums via one-hot matmuls in fp8 DoubleRow mode (two row
  tiles per matmul); counts via a ones-vector DoubleRow matmul. PSUM pass A
  covers d-chunks 0-2 + counts (8 banks), pass B covers d-chunk 3.
  One-hots built on DVE/GPSIMD (fp8, exact 0/1).

  ReduceScatter of [8 x (64 sums d-rows ; replicated counts)] x 1000 in
  bf16 (counts are small integers => bf16-exact): each core receives its
  64-row global-sums shard plus global counts. The sharded prototype
  update protos2 = 2*(A*p0 + B*sums) then runs 8-way parallel, and an
  fp8 AllGather distributes [protos2T-shard ; -p_sq/4 partial] — exactly
  the phase-2 matmul operand. The 8 p_sq partials are summed inside the
  phase-2 matmul itself (an extra DoubleRow matmul against constant pad
  weights). During the collectives: e_sq accumulated from bf16 rows (Act
  square+accum, loads on the Act DMA queue) and the transposed fp8
  embeddings DMA-loaded over the phase-1 row-major fp8 buffer.

  Phase 2: 2*emb@protos2T - p_sq via 3 DoubleRow fp8 matmuls per
  (tile, class-half); epilogue adds -e_sq as a per-partition bias
  (Act activation / DVE tensor_scalar, split) and emits bf16; the host
  casts bf16 -> f32.
"""

import os
import sys
from contextlib import ExitStack

for _p in ("/opt/trn_rl_repo", "/root/.axon_site/_ro/trn_rl_repo"):
    if os.path.isdir(_p):
        if _p not in sys.path:
            sys.path.insert(0, _p)
        break

import numpy as np
import ml_dtypes

import concourse.bass as bass
import concourse.mybir as mybir
import concourse.tile as tile
from concourse.bass_utils import run_bass_kernel_spmd

N, D, C = 65536, 512, 1000
W = 8                      # cores
NL = N // W                # rows per core
P = 128
KT = NL // P               # 64 row tiles per core
PAIRS = KT // 2            # 32 DoubleRow pairs
DC = D // P                # 4 contraction chunks of d
CH = ((0, 512), (512, 1000))   # class-axis halves (psum bank split)
F32 = mybir.dt.float32
BF16 = mybir.dt.bfloat16
F8 = mybir.dt.float8e4
ALU = mybir.AluOpType
ACTF = mybir.ActivationFunctionType
DR = mybir.MatmulPerfMode.DoubleRow

NP_BF16 = ml_dtypes.bfloat16
NP_F8 = mybir.dt.np(F8)

# Toggled by test.py for profiling runs.
PROFILE = False
TRACE_KWARGS = {}
LAST_RESULT = [None]

_built = [None]


def _split_waits(nc, cap=1):
    """Walrus in this container rejects >1 sync-wait per instruction.
    Move excess waits onto preceding same-engine NOPs (in-order engines,
    so semantics are preserved)."""
    n_new = 0
    for fn in nc.m.functions:
        for bb in fn.blocks:
            new_list = []
            for ins in bb.instructions:
                si = getattr(ins, "sync_info", None)
                if si is not None and si.on_wait and len(si.on_wait) > cap:
                    waits = list(si.on_wait)
                    keep, rest = waits[:cap], waits[cap:]
                    for i in range(0, len(rest), cap):
                        nop = mybir.InstNoOp(
                            name=f"I-waitsplit-{n_new}", ins=[], outs=[]
                        )
                        n_new += 1
                        nop.engine = ins.engine
                        nop.sync_info = mybir.SyncInfo(
                            on_wait=rest[i : i + cap], on_update=[]
                        )
                        new_list.append(nop)
                    si.on_wait = keep
                new_list.append(ins)
            bb.instructions = new_list
    return n_new


# e_sq chunk assignment, interleaved D/A so the streaming input pool
# recycles evenly: DVE (fused tensor_tensor_reduce), Act (square+accum).
E_CHUNK = 4
N_CHUNKS = KT // E_CHUNK
# Act chunks first: Act is idle during phase 1 (DVE owns the one-hot
# feed), and the trailing DVE chunks then land inside the collective
# window once DVE's feed work has drained. Act chunks are loaded from
# the Act DMA queue (self-paced); DVE chunks from the Pool queue so
# they don't serialize behind Act's compute.
E_ORDER = "AAAAAAAAAAAAAAAA"  # per-chunk engine (ttr is not codegen-safe)


def _build():
    nc = bass.Bass()
    emb16_ext = nc.declare_dram_parameter("emb16", [NL, D], BF16, isOutput=False)
    # partition-major fp8 layouts with DoubleRow weight pairs adjacent:
    # emb8[p, t, dc, i, m] = emb[(2t+i)*128+p, dc*128+m]
    # embT8[p, j, nt, i, m] = emb[nt*128+m, (2j+i)*128+p]
    emb8_ext = nc.declare_dram_parameter("emb8", [P, KT * D], F8, isOutput=False)
    embT8_ext = nc.declare_dram_parameter("embT8", [P, KT * D], F8,
                                          isOutput=False)
    yf_ext = nc.declare_dram_parameter("yf", [P, KT], F32, isOutput=False)
    counter_ext = nc.declare_dram_parameter("counter", [C], F32, isOutput=False)
    p0Ts_ext = nc.declare_dram_parameter("p0Ts", [D // W, C], BF16,
                                         isOutput=False)
    out_ext = nc.declare_dram_parameter("out", [NL, C], BF16, isOutput=True)

    with tile.TileContext(nc) as tc, ExitStack() as es:
        cpool = es.enter_context(tc.tile_pool(name="const", bufs=1))
        in_pool = es.enter_context(tc.tile_pool(name="inp", bufs=4))
        sq_pool = es.enter_context(tc.tile_pool(name="sq", bufs=3))
        pr_pool = es.enter_context(tc.tile_pool(name="pr", bufs=2))
        out_pool = es.enter_context(tc.tile_pool(name="outp", bufs=4))
        dram = es.enter_context(tc.tile_pool(name="dram", bufs=1, space="DRAM"))

        # ---- persistent SBUF ----
        iota = cpool.tile([P, C], F32, name="iota")
        nc.gpsimd.iota(
            iota[:], pattern=[[1, C]], base=0, channel_multiplier=0,
            allow_small_or_imprecise_dtypes=True,
        )
        ones_col = cpool.tile([P, 1], BF16, name="onesc")
        nc.vector.memset(ones_col[:], 1.0)
        ones_row = cpool.tile([1, P], BF16, name="onesr")
        nc.vector.memset(ones_row[:], 1.0)
        # dual-fp8 LdWeights requires a 16-wide stationary tile: the counts
        # matmul uses 16 duplicate ones columns (psum rows 1-15 are unused)
        ones2 = cpool.tile([P, 2 * 16], F8, name="ones2")
        nc.vector.memset(ones2[:], 1.0)
        padT = cpool.tile([P, 2 * P], F8, name="padT")
        nc.vector.memset(padT[:], 0.0)
        # (k=0..7, i=0) slots -> weight 1: sums the 8 gathered -p_sq partials
        nc.vector.memset(padT[0:W, 0:P], 1.0)

        y_sb = cpool.tile([P, KT], F32, name="y")
        nc.sync.dma_start(y_sb[:], yf_ext[:])
        ctr_row = cpool.tile([1, C], F32, name="ctr")
        nc.sync.dma_start(ctr_row[:], counter_ext[None, :])

        # emb8 (phase-1 row-major fp8) and embT8 (phase-2 transposed fp8)
        # share one buffer: embT8 is DMA-loaded after the last phase-1
        # matmul reads emb8 (the load overlaps the collective).
        ebuf = cpool.tile([P, KT * D], F8, name="ebuf")
        emb8 = ebuf
        embT8 = ebuf
        LCH = 2  # pairs per load DMA
        for g in range(PAIRS // LCH):
            s = slice(g * LCH * 2 * D, (g + 1) * LCH * 2 * D)
            nc.sync.dma_start(emb8[:, s], emb8_ext[:, s])

        # this core's 64-row shard of prototypes^T (used after the RS)
        p0Ts_sb = cpool.tile([64, C], BF16, name="p0Ts")
        nc.sync.dma_start(p0Ts_sb[:], p0Ts_ext[:, :])

        oh8 = cpool.tile([P, KT * C], F8, name="oh8")
        esq = cpool.tile([P, KT], F32, name="esq")
        esqn = cpool.tile([P, KT], F32, name="esqn")
        sums16 = cpool.tile([P, DC * C], BF16, name="sums16")
        counts16 = cpool.tile([1, C], BF16, name="counts16")
        counts8 = cpool.tile([W, C], BF16, name="counts8")
        p2x = cpool.tile([P, 6 * C], F8, name="p2x")
        nc.vector.memset(p2x[:, 4 * C : 6 * C], 0.0)  # pad rows for psq fold
        A_b = cpool.tile([P, C], BF16, name="Ab")
        B_b = cpool.tile([P, C], BF16, name="Bb")
        t1s = cpool.tile([P, C], BF16, name="t1s")
        t2s = cpool.tile([P, C], BF16, name="t2s")

        emb8r = emb8.rearrange("p (t dc i m) -> p t dc i m", t=PAIRS, dc=DC,
                               i=2)
        oh8r = oh8.rearrange("p (kt c) -> p kt c", kt=KT)
        embT8r = embT8.rearrange("p (j nt i m) -> p j nt i m", j=2, nt=KT,
                                 i=2)
        p2xr = p2x.rearrange("p (b c) -> p b c", b=6)
        sums16r = sums16.rearrange("p (dc c) -> p dc c", dc=DC)
        ones2r = ones2.rearrange("p (i m) -> p i m", i=2)
        padTr = padT.rearrange("p (i m) -> p i m", i=2)

        # ---- one-hots, split DVE/GPSIMD to match the pass-A matmul rate
        # (Pool takes odd tiles below kt=52; its tail would otherwise lag) ----
        for kt in range(KT):
            ohd = oh8[:, kt * C : (kt + 1) * C]
            eng = nc.gpsimd if (kt % 2 == 1 and kt < 52) else nc.vector
            eng.tensor_scalar(
                ohd, iota[:], y_sb[:, kt : kt + 1], None, ALU.is_equal
            )

        # ---- phase 1 matmuls: pass A (dc 0-2 + counts), pass B (dc 3) ----
        with tc.tile_pool(name="psA", bufs=1, space="PSUM") as psA:
            sA = [psA.tile([P, 1024], F32, tag=f"sA{dc}", name=f"sA{dc}")
                  for dc in range(3)]
            for t in range(PAIRS):
                st, sp = (t == 0), (t == PAIRS - 1)
                for dc in range(3):
                    for c0, c1 in CH:
                        nc.tensor.matmul(
                            sA[dc][:, c0:c1],
                            emb8r[:, t, dc, :, :],
                            oh8r[:, 2 * t : 2 * t + 2, c0:c1],
                            start=st, stop=sp, perf_mode=DR,
                        )
            # flush pass A (bf16 for the collective) on DVE
            for dc in range(3):
                nc.vector.tensor_copy(
                    out=sums16[:, dc * C : (dc + 1) * C], in_=sA[dc][:, 0:C]
                )

        # counter-only coefficient rows (emitted after the pass-A flush so
        # they don't delay the matmul feed; they run during pass B):
        # rt = 1/(counter+1);  preA = 2*(counter*rt - 1);  preB = 2*rt
        preA = cpool.tile([1, C], F32, name="preA")
        preB = cpool.tile([1, C], F32, name="preB")
        nc.vector.tensor_scalar(preB[:], ctr_row[:], 1.0, None, ALU.add)
        nc.vector.reciprocal(preB[:], preB[:])
        nc.vector.tensor_tensor(out=preA[:], in0=ctr_row[:], in1=preB[:],
                                op=ALU.mult)
        nc.vector.tensor_scalar(preA[:], preA[:], 1.0, None, ALU.subtract)
        nc.vector.tensor_scalar(preA[:], preA[:], 2.0, None, ALU.mult)
        nc.vector.tensor_scalar(preB[:], preB[:], 2.0, None, ALU.mult)

        # ---- ReduceScatter + sharded prototype math + AllGather ----
        # Each core owns one 64-row shard of the d axis. The RS input packs,
        # per shard, [64 sums rows ; local counts] so the reduction delivers
        # global counts to every core alongside its shard. After local
        # prototype math, the AllGather distributes [protos2T-shard ;
        # -p_sq partial] in fp8 (the phase-2 matmul operand, ready to use).
        SH = 65
        cc_rs_in = dram.tile([W * SH, C], BF16, tag="rsi", name="rsi")
        cc_rs_out = dram.tile([SH, C], BF16, tag="rso", name="rso")
        cc_ag_in = dram.tile([SH, C], F8, tag="agi", name="agi")
        cc_ag_out = dram.tile([W * SH, C], F8, tag="ago", name="ago",
                              addr_space="Shared")
        for i in range(6):
            nc.sync.dma_start(
                cc_rs_in[i * SH : i * SH + 64, :],
                sums16r[64 * (i % 2) : 64 * (i % 2) + 64, i // 2, :],
            )

        with tc.tile_pool(name="psB", bufs=1, space="PSUM") as psB:
            sB = psB.tile([P, 1024], F32, tag="sB", name="sB")
            cnt = psB.tile([16, 1024], F32, tag="cnt", name="cnt")
            for t in range(PAIRS):
                st, sp = (t == 0), (t == PAIRS - 1)
                for c0, c1 in CH:
                    nc.tensor.matmul(
                        sB[:, c0:c1],
                        emb8r[:, t, 3, :, :],
                        oh8r[:, 2 * t : 2 * t + 2, c0:c1],
                        start=st, stop=sp, perf_mode=DR,
                    )
                for c0, c1 in CH:
                    nc.tensor.matmul(
                        cnt[0:16, c0:c1],
                        ones2r[:, :, :],
                        oh8r[:, 2 * t : 2 * t + 2, c0:c1],
                        start=st, stop=sp, perf_mode=DR,
                    )
            nc.vector.tensor_copy(out=counts16[:], in_=cnt[0:1, 0:C])
            nc.vector.tensor_copy(out=sums16[:, 3 * C : 4 * C], in_=sB[:, 0:C])

            # replicate local counts across 8 partitions (ones outer product)
            cnt8ps = psB.tile([W, 1024], F32, tag="cnt8", name="cnt8")
            for c0, c1 in CH:
                nc.tensor.matmul(
                    cnt8ps[:, c0:c1], ones_row[0:1, 0:W], counts16[:, c0:c1],
                    start=True, stop=True,
                )
            nc.vector.tensor_copy(out=counts8[:], in_=cnt8ps[:, 0:C])

        for i in (6, 7):
            nc.sync.dma_start(
                cc_rs_in[i * SH : i * SH + 64, :],
                sums16r[64 * (i % 2) : 64 * (i % 2) + 64, i // 2, :],
            )
        nc.sync.dma_start(
            cc_rs_in.rearrange("(i r) c -> i r c", r=SH)[:, 64:65, :],
            counts8[:, None, :],
        )
        nc.gpsimd.collective_compute(
            "ReduceScatter", ALU.add,
            replica_groups=[list(range(W))],
            ins=[cc_rs_in[:, :]], outs=[cc_rs_out[:, :]],
        )

        # ---- e_sq from bf16 rows + embT8 load, all during the collective.
        # All loads issued from the Act DMA queue: SP parks on the
        # collective's results and the collective freezes the Pool queue
        # for its whole duration, so neither may carry these loads.
        for g in range(N_CHUNKS):
            et = in_pool.tile([P, E_CHUNK * D], BF16, tag="et", name="et")
            dma_eng = nc.scalar
            dma_eng.dma_start(
                et.rearrange("p (j d) -> p j d", j=E_CHUNK),
                emb16_ext.rearrange("(j p) d -> p j d", p=P)[
                    :, g * E_CHUNK : (g + 1) * E_CHUNK, :
                ],
            )
            for j in range(E_CHUNK):
                kt = g * E_CHUNK + j
                ets = et[:, j * D : (j + 1) * D]
                scr = sq_pool.tile([P, D], BF16, tag="scr", name="scr")
                eng = E_ORDER[g]
                if eng == "D":
                    nc.vector.tensor_tensor_reduce(
                        out=scr[:], in0=ets, in1=ets, scale=1.0, scalar=0.0,
                        op0=ALU.mult, op1=ALU.add,
                        accum_out=esq[:, kt : kt + 1],
                    )
                else:
                    nc.scalar.activation(
                        scr[:], ets, ACTF.Square,
                        accum_out=esq[:, kt : kt + 1],
                    )
        # transposed fp8 embeddings overwrite emb8 (WAR on phase-1 matmuls);
        # issued from the Act queue (after its e_sq work) so the transfer
        # lands inside the collective window without delaying the staging.
        nc.scalar.dma_start(embT8[:, :], embT8_ext[:, :])

        # RS results back (SP queue; parks until the collective finishes).
        # Counts row first: the coefficient rows depend only on counts.
        shard_s = cpool.tile([64, C], BF16, name="shards")
        nc.sync.dma_start(counts16[:], cc_rs_out[64:65, :])
        nc.sync.dma_start(shard_s[:], cc_rs_out[0:64, :])

        # ---- negate e_sq (phase-2 bias) ----
        nc.vector.tensor_scalar(esqn[:], esq[:], -1.0, None, ALU.mult)

        # ---- per-class coefficients:  protos2 = 2A*p0 + 2B*sums
        # 2A = 2 + rep*preA ;  2B = rep * (preB * 1/max(counts,1))
        countsf = cpool.tile([1, C], F32, name="countsf")
        rep = cpool.tile([1, C], F32, name="rep")
        tmp1 = cpool.tile([1, C], F32, name="tmp1")
        tmp2 = cpool.tile([1, C], F32, name="tmp2")
        A_row = cpool.tile([1, C], BF16, name="Arow")
        B_row = cpool.tile([1, C], BF16, name="Brow")
        nc.vector.tensor_copy(out=countsf[:], in_=counts16[:])
        nc.vector.tensor_scalar(rep[:], countsf[:], 0.0, None, ALU.is_gt)
        nc.vector.tensor_scalar(tmp1[:], countsf[:], 1.0, None, ALU.max)
        nc.vector.reciprocal(tmp1[:], tmp1[:])
        nc.vector.tensor_tensor(out=tmp2[:], in0=preB[:], in1=tmp1[:],
                                op=ALU.mult)
        nc.vector.tensor_tensor(out=B_row[:], in0=tmp2[:], in1=rep[:],
                                op=ALU.mult)
        nc.vector.tensor_tensor(out=tmp2[:], in0=preA[:], in1=rep[:],
                                op=ALU.mult)
        nc.vector.tensor_scalar(A_row[:], tmp2[:], 2.0, None, ALU.add)

        with tc.tile_pool(name="psM", bufs=1, space="PSUM") as psM:
            # broadcast A,B down partitions via ones outer product
            for row, dstb in ((B_row, B_b), (A_row, A_b)):
                ob = psM.tile([64, 1024], F32, tag="ob", bufs=2, name="ob")
                for c0, c1 in CH:
                    nc.tensor.matmul(
                        ob[:, c0:c1], ones_row[0:1, 0:64], row[:, c0:c1],
                        start=True, stop=True,
                    )
                nc.vector.tensor_copy(out=dstb[0:64, :], in_=ob[:, 0:C])

            # this core's 64-row protos2T shard (fp8) + its -p_sq partial
            p2sh = cpool.tile([64, C], F8, name="p2sh")
            nc.vector.tensor_tensor(out=t1s[0:64, :], in0=p0Ts_sb[:],
                                    in1=A_b[0:64, :], op=ALU.mult)
            nc.vector.tensor_tensor(out=t2s[0:64, :], in0=shard_s[:],
                                    in1=B_b[0:64, :], op=ALU.mult)
            nc.vector.tensor_tensor(out=p2sh[:], in0=t1s[0:64, :],
                                    in1=t2s[0:64, :], op=ALU.add)
            sqs = pr_pool.tile([64, C], BF16, tag="sqs", name="sqs")
            nc.vector.tensor_tensor(out=sqs[:], in0=p2sh[:],
                                    in1=p2sh[:], op=ALU.mult)
            psqps = psM.tile([1, 1024], F32, tag="psq", name="psq")
            for c0, c1 in CH:
                nc.tensor.matmul(
                    psqps[:, c0:c1], ones_col[0:64, :], sqs[:, c0:c1],
                    start=True, stop=True,
                )
            psqn = tmp1  # coefficient scratch rows are dead by now
            nc.vector.tensor_scalar(psqn[:], psqps[0:1, 0:C], -0.25, None,
                                    ALU.mult)
            psq8 = cpool.tile([1, C], F8, name="psq8")
            nc.vector.tensor_copy(out=psq8[:], in_=psqn[:])

        # AllGather [protos2T-shard ; -p_sq partial] (fp8)
        nc.sync.dma_start(cc_ag_in[0:64, :], p2sh[:])
        nc.sync.dma_start(cc_ag_in[64:65, :], psq8[:])
        nc.gpsimd.collective_compute(
            "AllGather", ALU.bypass,
            replica_groups=[list(range(W))],
            ins=[cc_ag_in[:, :]], outs=[cc_ag_out[:, :]],
        )
        # assemble the phase-2 operand: p2x[p, dc, :] holds d = dc*128+p,
        # and the 8 -p_sq partials land on partitions 0-7 of pad block 4
        for dc in range(DC):
            nc.sync.dma_start(
                p2xr[0:64, dc, :], cc_ag_out[2 * dc * SH : 2 * dc * SH + 64, :]
            )
            nc.sync.dma_start(
                p2xr[64:P, dc, :],
                cc_ag_out[(2 * dc + 1) * SH : (2 * dc + 1) * SH + 64, :],
            )
        nc.sync.dma_start(
            p2x[0:W, 4 * C : 5 * C],
            cc_ag_out.rearrange("(i r) c -> i r c", r=SH)[:, 64, :],
        )

        # ---- phase 2: out = 2*emb@protos2T - p_sq - e_sq ----
        OB = 2  # output tiles per DMA
        with tc.tile_pool(name="ps2", bufs=4, space="PSUM") as ps2:
            for nt in range(KT):
                if nt % OB == 0:
                    ot = out_pool.tile([P, OB * C], BF16, tag="ot", name="ot")
                # per-half psum groups: each class-half closes and drains
                # independently, halving the psum-token recycle latency
                crh = [ps2.tile([P, 512], F32, tag=f"cr{ci}", name=f"cr{ci}")
                       for ci in range(2)]
                ots = ot[:, (nt % OB) * C : (nt % OB + 1) * C]
                for ci, (c0, c1) in enumerate(CH):
                    cw = c1 - c0
                    cr = crh[ci]
                    for j in range(2):
                        nc.tensor.matmul(
                            cr[:, 0:cw],
                            embT8r[:, j, nt, :, :],
                            p2xr[:, 2 * j : 2 * j + 2, c0:c1],
                            start=(j == 0), stop=False, perf_mode=DR,
                        )
                    nc.tensor.matmul(
                        cr[:, 0:cw],
                        padTr[:, :, :],
                        p2xr[:, 4:6, c0:c1],
                        start=False, stop=True, perf_mode=DR,
                    )
                    # epilogue: Act drains half A, DVE half B, in parallel
                    if ci == 0:
                        nc.scalar.activation(
                            ots[:, c0:c1], cr[:, 0:cw], ACTF.Identity,
                            bias=esqn[:, nt : nt + 1], scale=1.0,
                        )
                    else:
                        nc.vector.tensor_scalar(
                            ots[:, c0:c1], cr[:, 0:cw],
                            esqn[:, nt : nt + 1], None, ALU.add
                        )
                if nt % OB == OB - 1:
                    nc.sync.dma_start(
                        out_ext.rearrange("(j p) c -> p j c", p=P)[
                            :, nt - OB + 1 : nt + 1, :
                        ],
                        ot.rearrange("p (j c) -> p j c", j=OB),
                    )

    _split_waits(nc)
    return nc


def kernel(embeddings, prototypes, counter, y_true):
    embeddings = np.ascontiguousarray(np.asarray(embeddings, dtype=np.float32))
    prototypes = np.ascontiguousarray(np.asarray(prototypes, dtype=np.float32))
    counter_f = np.ascontiguousarray(np.asarray(counter, dtype=np.float32))
    y = np.asarray(y_true)

    if _built[0] is None:
        _built[0] = _build()
    nc = _built[0]

    p0T16 = np.ascontiguousarray(prototypes.T).astype(NP_BF16)  # [512, 1000]
    in_maps = []
    for i in range(W):
        sl = slice(i * NL, (i + 1) * NL)
        e = embeddings[sl]
        e8 = e.astype(NP_F8)
        # emb8[p, t, dc, i, m]; embT8[p, j, nt, i, m] (DoubleRow pairs adjacent)
        e8v = e8.reshape(PAIRS, 2, P, DC, P)
        e8p = np.ascontiguousarray(e8v.transpose(2, 0, 3, 1, 4)).reshape(P, KT * D)
        eTv = np.ascontiguousarray(e8.T).reshape(2, 2, P, KT, P)
        eTp = np.ascontiguousarray(eTv.transpose(2, 0, 3, 1, 4)).reshape(P, KT * D)
        y_loc = y[sl].astype(np.float32)
        yf = np.ascontiguousarray(y_loc.reshape(KT, P).T)
        in_maps.append(
            {
                "emb16": e.astype(NP_BF16),
                "emb8": e8p,
                "embT8": eTp,
                "yf": yf,
                "counter": counter_f,
                "p0Ts": np.ascontiguousarray(p0T16[i * (D // W) : (i + 1) * (D // W)]),
            }
        )

    res = run_bass_kernel_spmd(
        nc, in_maps, list(range(W)), trace=PROFILE, **TRACE_KWARGS
    )
    LAST_RESULT[0] = res
    out = np.concatenate(
        [np.asarray(res.results[i]["out"]) for i in range(W)], axis=0
    )
    return out.astype(np.float32)
